# revision 24
# baseline (speedup 1.0000x reference)
"""Bilinear interaction kernel for Trainium2 (8 NeuronCores, SPMD).

Problem: inputs (32, 4096, 1, 64) f32 stacked field embeddings,
W (496, 64, 64) one bilinear weight per field pair (i<j).
out[b, p] = x_i[b] @ W_p @ x_j[b]   -> (4096, 496) f32.

Strategy (data-parallel over batch per the sharding hint): batch 4096 ->
8 cores x 512 rows; W replicated. Per core, per 128-row b-tile:

 stage 1 (PE, bf16):  T[b, (j,l)] = x_i[b,:] @ W_i-block   (PSUM f32)
 stage 2 (mult):      M[b, (j,l)] = T * xn[b, (j,l)]       (bf16 SBUF)
 stage 3 (reduce):    out[b, p]   = sum_l M[b, (p,l)]

Stages 2+3 are the elementwise bottleneck (B*P*K = 16.25M elems/core), so
they are load-balanced across all four non-PE-capable paths:
  - mult: DVE fused from PSUM (1x f32), or ACT extract (PSUM->bf16 SBUF)
    followed by a 2x bf16 DVE mult or a GPSIMD mult.
  - reduce: PE accumulating identity-matmuls into a PSUM acc bank, or a
    log2 tree of bf16 adds on DVE / GPSIMD.
Routing fractions are tunable (BIL_MW / BIL_RED env).

walrus allows ONE sync wait per instruction, so: every cross-engine
dependency that would add a second wait is first absorbed into the
consumer engine's vector clock by a cheap "touch" op (tiny matmul into a
write-only psum sink / 1-elem copy), mirroring the proven baseline
pattern.
"""

import os
import sys

import numpy as np

sys.path.insert(0, "/opt/trn_rl_repo")

import ml_dtypes

import concourse.bass as bass
import concourse.tile as tile
from concourse import mybir
from concourse.bass_utils import run_bass_kernel_spmd
from concourse.tile import ScopedClock


def _split_drain_and_barrier(self, tick_clock, wait_clock):
    """Replacement for TileContext._drain_and_barrier: walrus codegen
    accepts only one sync wait per instruction in this toolchain, but the
    kernel-tail drain collects one wait per active processor. Emit one
    drain per wait instead."""
    drains = [self.nc.sync.drain() for _ in range(20)]
    wait_clock.add_sem_waits(
        drains[-1].ins, ScopedClock({None: tick_clock.global_clock})
    )
    si = drains[-1].ins.sync_info
    ow = list(si.on_wait) if si is not None and si.on_wait else []
    if len(ow) > 1:
        for d, w in zip(drains[:-1], ow[:-1]):
            d.ins.sync_info = mybir.SyncInfo(on_wait=[w], on_update=[])
        drains[-1].ins.sync_info = mybir.SyncInfo(
            on_wait=[ow[-1]],
            on_update=list(si.on_update) if si.on_update else [],
        )

    self.nc.all_engine_barrier()
    assert self.sems is not None
    popped = self.nc._tile_sem_poison_stack.pop()
    assert popped is self._sem_poison
    self.nc.clear_and_free_semaphores(list(self.sems.allocated().values()))
    self.nc.all_engine_barrier()


tile.TileContext._drain_and_barrier = _split_drain_and_barrier

NF = 32          # fields
B = 4096         # total batch
K = 64           # embedding dim
P = NF * (NF - 1) // 2   # 496 pairs
NCORES = 8
BC = B // NCORES          # 512 rows per core
BT = 128                  # batch tile (partition dim)
NBT = BC // BT            # 4 batch tiles per core
F32 = mybir.dt.float32
BF16 = mybir.dt.bfloat16
CHUNK = 512
SPAN = 62                 # pairs per reduce span; 496 = 8 * 62
NSPAN = P // SPAN

# ---- tunables ---------------------------------------------------------------
# mult route weights (F = DVE fused-from-psum, D = ACT extract + DVE bf16
# mult, G = ACT extract + GPSIMD mult), cycled over units by largest-
# remainder so fractions hold.
_MW = tuple(int(x) for x in os.environ.get("BIL_MW", "4,3,5").split(","))
# reduce route per span (P = PE identity-matmul, D = DVE tree, G = GP tree)
_RED = os.environ.get("BIL_RED", "PDPGPDPG")
assert len(_RED) == NSPAN
_LAG = int(os.environ.get("BIL_LAG", "1"))   # stage1 -> consume unit lag
_DRAIN = int(os.environ.get("BIL_DRAIN", "7"))  # PE fifo ops per unit step
_TSB_BUFS = 6

# pair-group column offsets (pairs ordered like itertools.combinations)
_GRP_OFF = [0] * NF
for _i in range(1, NF):
    _GRP_OFF[_i] = _GRP_OFF[_i - 1] + (NF - _i)

# ---- field -> (xt slot, half) packing ---------------------------------------
# top half (partitions 0:64): fields 0-7 and 24-31 (col-balanced with the
# bottom half: both hold 15872 T columns)


def _field_loc(i: int):
    if i < 8:
        return i, 0          # slot, top half
    if i < 24:
        return i - 8, 1      # bottom half
    return i - 16, 0         # top half

_XTP_FIELD = np.zeros((2, 16), dtype=np.int64)   # [half, slot] -> field
for _i in range(NF):
    _sl, _hf = _field_loc(_i)
    _XTP_FIELD[_hf, _sl] = _i

# ---- chunks, units, W blocks ------------------------------------------------
_CHUNKS = []     # (i, off, w) in i-ascending (== pair) order
for _i in range(NF - 1):
    _ncols = (NF - 1 - _i) * K
    for _off in range(0, _ncols, CHUNK):
        _CHUNKS.append((_i, _off, min(CHUNK, _ncols - _off)))

# units: <=2 consecutive same-group chunks -> one psum tile + one mult span
_UNITS = []      # (i, off0, [w1, (w2)])
_k = 0
while _k < len(_CHUNKS):
    _i, _off, _w = _CHUNKS[_k]
    ws = [_w]
    if _k + 1 < len(_CHUNKS) and _CHUNKS[_k + 1][0] == _i:
        ws.append(_CHUNKS[_k + 1][2])
    _UNITS.append((_i, _off, ws))
    _k += len(ws)

# W blocks: pair k-th top-half chunk with k-th bottom-half chunk (both
# sequences in i-order), so DMA delivery order roughly matches first use.
_top = [c for c in _CHUNKS if _field_loc(c[0])[1] == 0]
_bot = [c for c in _CHUNKS if _field_loc(c[0])[1] == 1]
assert len(_top) == len(_bot) == 38
_WBLK = []
_WCOL = []
_c = 0
for _j in range(38):
    _w = max(_top[_j][2], _bot[_j][2])
    _WBLK.append((_top[_j], _bot[_j], _w))
    _WCOL.append(_c)
    _c += _w
_WCOLS = _c

_CHUNK_LOC = {}
for _j, (_ct, _cb, _w) in enumerate(_WBLK):
    _CHUNK_LOC[(_ct[0], _ct[1])] = (_j, 0)
    _CHUNK_LOC[(_cb[0], _cb[1])] = (_j, 1)

# W DMA groups: consecutive block ranges, ~5 blocks each
_WGRP = []
_j = 0
while _j < 38:
    _je = min(_j + 5, 38)
    _WGRP.append((_j, _je))
    _j = _je


def _mult_routes(n):
    """largest-remainder cycle of F/D/G with weights _MW."""
    tot = sum(_MW)
    routes = []
    cnt = [0.0, 0.0, 0.0]
    for k in range(n):
        # target counts after k+1 units
        best, bestgap = 0, -1e9
        for r in range(3):
            gap = (k + 1) * _MW[r] / tot - cnt[r]
            if gap > bestgap:
                best, bestgap = r, gap
        cnt[best] += 1
        routes.append("FDG"[best])
    return routes


_MROUTE = _mult_routes(len(_UNITS))


def _build_module() -> bass.Bass:
    nc = bass.Bass()
    xnb = nc.declare_dram_parameter("xnb", [BC, NF * K], BF16, isOutput=False)
    xtp = nc.declare_dram_parameter("xtp", [BT, NBT, 16, BT], BF16, isOutput=False)
    wt = nc.declare_dram_parameter("wt", [BT, _WCOLS], BF16, isOutput=False)
    ident = nc.declare_dram_parameter("ident", [BT, BT], BF16, isOutput=False)
    outs = [
        nc.declare_dram_parameter(f"out{t}", [BT, P], F32, isOutput=True)
        for t in range(NBT)
    ]

    with tile.TileContext(nc) as tc:
        with (
            tc.tile_pool(name="wtp", bufs=1) as wtp,
            tc.tile_pool(name="xtpp", bufs=1) as xtpp,
            tc.tile_pool(name="xnbp", bufs=1) as xnbp,
            tc.tile_pool(name="mpP", bufs=3) as mpP,
            tc.tile_pool(name="mpD", bufs=3) as mpD,
            tc.tile_pool(name="mpG", bufs=3) as mpG,
            tc.tile_pool(name="tsbD", bufs=_TSB_BUFS) as tsbDp,
            tc.tile_pool(name="tsbG", bufs=_TSB_BUFS) as tsbGp,
            tc.tile_pool(name="trD", bufs=2) as trDp,
            tc.tile_pool(name="trG", bufs=2) as trGp,
            tc.tile_pool(name="outp", bufs=1) as outp,
            tc.tile_pool(name="psA", bufs=3, space=bass.MemorySpace.PSUM) as psA,
            tc.tile_pool(name="psB", bufs=2, space=bass.MemorySpace.PSUM) as psB,
            tc.tile_pool(name="accp", bufs=1, space=bass.MemorySpace.PSUM) as accp,
        ):
            junk = wtp.tile([BT, 256], F32, tag="junk")
            junka = wtp.tile([BT, 256], F32, tag="junka")
            junkg = wtp.tile([BT, 256], F32, tag="junkg")
            _tc_ = {"d": 0, "a": 0, "g": 0}

            def pe_touch(ap2d):
                # zero-cost PE clock absorber: a standalone LDWEIGHTS reads
                # the (bf16) tile; every real matmul self-loads its weights,
                # so clobbering the stationary register is harmless.
                nc.tensor.ldweights(ap2d[:, 0:1])

            def dve_touch(ap2d):
                c = _tc_["d"] % 256
                _tc_["d"] += 1
                nc.vector.tensor_copy(junk[0:1, c:c + 1], ap2d[0:1, 0:1])

            def act_touch(ap2d):
                c = _tc_["a"] % 256
                _tc_["a"] += 1
                nc.scalar.copy(junka[0:1, c:c + 1], ap2d[0:1, 0:1])

            def gp_touch(ap2d):
                c = _tc_["g"] % 256
                _tc_["g"] += 1
                nc.gpsimd.tensor_copy(junkg[0:1, c:c + 1], ap2d[0:1, 0:1])

            def touch_on(eng, ap2d):
                if eng == "D":
                    dve_touch(ap2d)
                elif eng == "G":
                    gp_touch(ap2d)
                elif eng == "A":
                    act_touch(ap2d)
                else:
                    pe_touch(ap2d)

            # ---- persistent tiles & DMA prologue ----------------------------
            wt_sb = wtp.tile([BT, _WCOLS], BF16, tag="wt_sb")
            ident_sb = wtp.tile([BT, BT], BF16, tag="ident_sb")
            acc = accp.tile([BT, P], F32, tag="acc")

            # eat the one-time ACT table load at t=0 (before any deps)
            nc.vector.memset(junka[0:1, 0:2], 0)
            nc.scalar.copy(junka[0:1, 0:1], junka[0:1, 1:2])
            # pre-warm the PE p-state during the DMA prologue: ~4us of dummy
            # matmuls so real stage-1 runs at full clock from the start.
            junkb = wtp.tile([BT, 64], BF16, tag="junkb")
            nc.vector.memset(junkb[:], 0)
            for _ in range(72):
                nc.tensor.matmul(
                    acc[0:1, 0:64], junkb[:, 0:1], junkb[:, 0:64],
                    start=True, stop=True,
                )
            xt_tiles = [None] * NBT
            xnb_tiles = [None] * NBT

            def load_xt(t):
                xg = xtpp.tile([BT, 16, BT], BF16, tag=f"xt{t}", name="xg")
                nc.sync.dma_start(xg[:], xtp[:, t])
                pe_touch(xg[:, 0, :])
                xt_tiles[t] = xg

            def load_xnb(t):
                xb = xnbp.tile([BT, NF * K], BF16, tag=f"xnb{t}", name="xb")
                nc.sync.dma_start(xb[:], xnb[t * BT:(t + 1) * BT, :])
                dve_touch(xb)
                gp_touch(xb)
                xnb_tiles[t] = xb

            def load_wgrp(gi):
                j0, je = _WGRP[gi]
                c0 = _WCOL[j0]
                c1 = _WCOL[je - 1] + _WBLK[je - 1][2]
                nc.sync.dma_start(wt_sb[:, c0:c1], wt[:, c0:c1])
                pe_touch(wt_sb[:, c0:c1])

            load_xt(0)
            load_wgrp(0)
            load_xnb(0)
            nc.sync.dma_start(ident_sb[:], ident[:])
            pe_touch(ident_sb)
            load_wgrp(1)
            load_wgrp(2)
            load_xt(1)
            load_xnb(1)
            load_wgrp(3)
            load_wgrp(4)
            load_xt(2)
            load_xnb(2)
            load_wgrp(5)
            load_wgrp(6)
            load_xt(3)
            load_xnb(3)
            load_wgrp(7)

            # per-route history for pool-wrap touch absorption
            outsb_hist = {"P": [], "D": [], "G": []}   # out_sb slices per span
            tsb_alloc = {"D": 0, "G": 0}

            for t in range(NBT):
                xt_sb = xt_tiles[t]
                xb = xnb_tiles[t]
                out_sb = outp.tile([BT, P], F32, tag=f"osb{t}", name="out_sb")

                # span state: [m_tile, writers [(eng, slice)], filled, p_idx]
                spans = [[None, [], 0, -1] for _ in range(NSPAN)]
                outsb_written = {"A": None, "D": None, "G": None}
                unit_ps = {}     # u_idx -> psum tile
                # PE reduce work is queued as closures and drained a few ops
                # at a time between stage-1 matmuls, so the in-order PE queue
                # never monopolizes 64-matmul bursts while DVE/ACT starve.
                pe_fifo = []
                msl_hist = {"D": [], "G": []}
                p_burst_done = []   # per P-tile-alloc: burst fully emitted?

                def fifo_drain(n=None):
                    k = len(pe_fifo) if n is None else min(n, len(pe_fifo))
                    for _ in range(k):
                        marker, fn = pe_fifo.pop(0)
                        fn()
                        if marker is not None:
                            p_burst_done[marker] = True

                def get_span_tile(s, eng, spans=spans):
                    st = spans[s]
                    if st[0] is None:
                        r = _RED[s]
                        pool = {"P": mpP, "D": mpD, "G": mpG}[r]
                        if r == "P":
                            # the pool slot this alloc reuses (3 allocs ago)
                            # must have its queued reduce burst emitted first
                            ab = len(p_burst_done)
                            if ab >= 3:
                                while not p_burst_done[ab - 3]:
                                    fifo_drain(1)
                            p_burst_done.append(False)
                        hist = outsb_hist[r]
                        if len(hist) >= 2:
                            touch_on(eng, hist[-2])
                            del hist[: len(hist) - 2]
                        st[0] = pool.tile([BT, SPAN, K], BF16, name="mspan")
                        st[3] = len(p_burst_done) - 1 if r == "P" else -1
                    return st[0]

                def emit_mult(eng, src_tile, src_c0, i, off0, p0, g,
                              t=t, xb=xb, spans=spans):
                    """multiply g pairs of T (at src_tile[:, src_c0:...])
                    into m-span tiles, splitting at span boundaries."""
                    done = 0
                    while done < g:
                        s = (p0 + done) // SPAN
                        loc = (p0 + done) - s * SPAN
                        take = min(g - done, SPAN - loc)
                        mt = get_span_tile(s, eng)
                        out_ap = mt[:, loc:loc + take, :].rearrange(
                            "p a b -> p (a b)"
                        )
                        o = done * K
                        in0 = src_tile[:, src_c0 + o: src_c0 + o + take * K]
                        in1 = xb[:, (i + 1) * K + off0 + o:
                                 (i + 1) * K + off0 + o + take * K]
                        if eng == "D":
                            nc.vector.tensor_mul(out_ap, in0, in1)
                        else:
                            nc.gpsimd.tensor_mul(out_ap, in0, in1)
                        msl = mt[0:64, loc, :]
                        msl_hist[eng].append(msl)
                        spans[s][1].append((eng, msl))
                        spans[s][2] += take
                        if spans[s][2] == SPAN:
                            emit_reduce(s)
                        done += take

                def emit_reduce(s, t=t, spans=spans, out_sb=out_sb):
                    r = _RED[s]
                    mt, writers, _, p_idx = spans[s]
                    osl = out_sb[:, s * SPAN:(s + 1) * SPAN]
                    if r == "P":
                        # queue the burst on the PE fifo (drained interleaved)
                        for eng, msl in writers:
                            pe_fifo.append(
                                (None, lambda msl=msl: pe_touch(msl))
                            )
                        for l in range(K):
                            pe_fifo.append((
                                None,
                                lambda l=l, mt=mt, s=s: nc.tensor.matmul(
                                    acc[:, s * SPAN:(s + 1) * SPAN],
                                    ident_sb[:, :],
                                    mt[:, :, l],
                                    start=(l == 0),
                                    stop=(l == K - 1),
                                ),
                            ))

                        def acc_copy(osl=osl, s=s):
                            nc.scalar.copy(
                                osl, acc[:, s * SPAN:(s + 1) * SPAN]
                            )
                            outsb_written["A"] = osl

                        pe_fifo.append((p_idx, acc_copy))
                    else:
                        veng = nc.vector if r == "D" else nc.gpsimd
                        pool = trDp if r == "D" else trGp
                        for eng, msl in writers:
                            if eng != r:
                                touch_on(r, msl)
                        cur = mt
                        width = K
                        while width > 2:
                            half = width // 2
                            nt = pool.tile([BT, SPAN, half], BF16, name="tr")
                            veng.tensor_add(
                                nt[:], cur[:, :, 0:half], cur[:, :, half:width]
                            )
                            cur = nt
                            width = half
                        veng.tensor_add(osl, cur[:, :, 0], cur[:, :, 1])
                        outsb_written[r] = osl
                    outsb_hist[r].append(osl)
                    spans[s][0] = None
                    spans[s][1] = []

                def emit_stage1(u_idx):
                    i, off0, ws = _UNITS[u_idx]
                    sl, hf = _field_loc(i)
                    pb = 64 * hf
                    route = _MROUTE[u_idx]
                    tiles = []
                    off = off0
                    if route == "F":
                        # chunk-granular psum (1 bank each) for pipeline depth
                        for w in ws:
                            j, half = _CHUNK_LOC[(i, off)]
                            assert half == hf
                            ps = psA.tile([BT, 512], F32, name="psa")
                            nc.tensor.matmul(
                                ps[:, :w],
                                xt_sb[pb:pb + 64, sl, :],
                                wt_sb[pb:pb + 64, _WCOL[j]:_WCOL[j] + w],
                                start=True,
                                stop=True,
                            )
                            tiles.append(ps)
                            off += w
                    else:
                        ps = psB.tile([BT, 1024], F32, name="psb")
                        col = 0
                        for w in ws:
                            j, half = _CHUNK_LOC[(i, off)]
                            assert half == hf
                            nc.tensor.matmul(
                                ps[:, col:col + w],
                                xt_sb[pb:pb + 64, sl, :],
                                wt_sb[pb:pb + 64, _WCOL[j]:_WCOL[j] + w],
                                start=True,
                                stop=True,
                            )
                            col += w
                            off += w
                        tiles.append(ps)
                    unit_ps[u_idx] = tiles

                def emit_consume(u_idx):
                    i, off0, ws = _UNITS[u_idx]
                    route = _MROUTE[u_idx]
                    W = sum(ws)
                    g = W // K
                    p0 = _GRP_OFF[i] + off0 // K
                    tiles = unit_ps.pop(u_idx)
                    if route == "F":
                        off = off0
                        pp = p0
                        for ps, w in zip(tiles, ws):
                            emit_mult("D", ps, 0, i, off, pp, w // K)
                            off += w
                            pp += w // K
                    else:
                        eng = "D" if route == "D" else "G"
                        tpool = tsbDp if eng == "D" else tsbGp
                        tsb_alloc[eng] += 1
                        if (tsb_alloc[eng] > _TSB_BUFS
                                and tsb_alloc[eng] % 3 == 0
                                and len(msl_hist[eng]) >= 3):
                            act_touch(msl_hist[eng][-3])
                            del msl_hist[eng][:-3]
                        tsb = tpool.tile([BT, 1024], BF16, name="tsb")
                        nc.scalar.copy(tsb[:, :W], tiles[0][:, :W])
                        emit_mult(eng, tsb, 0, i, off0, p0, g)

                # software-pipelined emission: consume trails produce by
                # _LAG units; PE reduce bursts drain interleaved.
                NU = len(_UNITS)
                for u in range(NU + _LAG):
                    if u >= _LAG:
                        emit_consume(u - _LAG)
                    if u < NU:
                        emit_stage1(u)
                    fifo_drain(_DRAIN)
                fifo_drain()

                # final output DMA (on gpsimd/SWDGE queue)
                for e in ("A", "D"):
                    if outsb_written[e] is not None:
                        gp_touch(outsb_written[e])
                nc.gpsimd.dma_start(outs[t][:], out_sb[:])
    return nc


_NC_CACHE: dict[str, bass.Bass] = {}


def _get_module() -> bass.Bass:
    if "nc" not in _NC_CACHE:
        _NC_CACHE["nc"] = _build_module()
    return _NC_CACHE["nc"]


def _make_in_maps(inputs: np.ndarray, W: np.ndarray):
    x = np.ascontiguousarray(np.asarray(inputs, dtype=np.float32)[:, :, 0, :])
    W = np.asarray(W, dtype=np.float32)

    # packed W: block j = [top chunk | bottom chunk] on partition halves
    wt_host = np.zeros((BT, _WCOLS), dtype=np.float32)
    wt_flat = np.ascontiguousarray(W.transpose(1, 0, 2)).reshape(K, P * K)
    for j, (ct, cb, w) in enumerate(_WBLK):
        for half, (i, off, cw) in ((0, ct), (1, cb)):
            base = _GRP_OFF[i] * K + off
            wt_host[64 * half: 64 * half + 64, _WCOL[j]: _WCOL[j] + cw] = \
                wt_flat[:, base: base + cw]
    wt_host = wt_host.astype(ml_dtypes.bfloat16)

    ident_host = np.eye(BT, dtype=ml_dtypes.bfloat16)

    in_maps = []
    for c in range(NCORES):
        xs = x[:, c * BC:(c + 1) * BC, :]                      # (32, 512, 64)
        xnb_host = np.ascontiguousarray(
            xs.transpose(1, 0, 2)
        ).reshape(BC, NF * K).astype(ml_dtypes.bfloat16)
        # xtp[p, t, slot, b-local]: p<64 top fields, p>=64 bottom, k = p % 64
        xtp_host = np.empty((BT, NBT, 16, BT), dtype=np.float32)
        xt_all = xs.transpose(2, 0, 1)                         # (64, 32, 512)
        for t in range(NBT):
            xtp_host[0:64, t] = xt_all[:, _XTP_FIELD[0], t * BT:(t + 1) * BT]
            xtp_host[64:128, t] = xt_all[:, _XTP_FIELD[1], t * BT:(t + 1) * BT]
        xtp_host = xtp_host.astype(ml_dtypes.bfloat16)
        in_maps.append({
            "xnb": xnb_host, "xtp": xtp_host, "wt": wt_host,
            "ident": ident_host,
        })
    return in_maps


def kernel(inputs: np.ndarray, W: np.ndarray) -> np.ndarray:
    in_maps = _make_in_maps(inputs, W)
    nc = _get_module()
    res = run_bass_kernel_spmd(nc, in_maps, list(range(NCORES))).results
    return np.concatenate(
        [r[f"out{t}"] for r in res for t in range(NBT)], axis=0
    )


def kernel_profiled(inputs: np.ndarray, W: np.ndarray, tmpdir: str | None = None):
    """Run with NTFF tracing; returns (output, BassKernelResults)."""
    in_maps = _make_in_maps(inputs, W)
    nc = _get_module()
    br = run_bass_kernel_spmd(
        nc, in_maps, list(range(NCORES)), trace=True, tmpdir=tmpdir
    )
    out = np.concatenate(
        [r[f"out{t}"] for r in br.results for t in range(NBT)], axis=0
    )
    return out, br
